# revision 1
# baseline (speedup 1.0000x reference)
"""TRN2 Bass/Tile kernel: 16-head MHA (N=2, S=2048, D=1024) on 8 NeuronCores.

Sharding (hardcoded): core c = 4*n + g runs batch n (data parallel, N=2) and
head group g (tensor parallel, 4 heads each).  Wq/Wk/Wv are column-sharded
[1024, 256], Wp row-sharded [256, 1024].  Each core produces a partial
projection [2048, 1024]; the host sums the 4 partials of each batch and adds
the (bv @ Wp + bp) terms (exact, since dropout is identity and the projection
is linear in bv).

Device-side dataflow per core (all matmuls bf16 with fp32 PSUM accumulation):
  - host hands the core its batch's activations pre-transposed x^T [1024,2048]
    (pure layout change of the shard); xq/xk load fp32 via HWDGE and cast to
    bf16 on DVE/ACT, xv loads through the (slower) gpsimd cast-DMA path so
    both DMA paths run concurrently
  - Q^T, K^T [256, 2048] computed with heads on partitions (head pairs share a
    128-partition chunk), V [2048, 256] computed straight with a ones column
    per head so the A@V matmul also accumulates the softmax denominator
  - scores are computed transposed (keys on partitions, queries on the free
    dim) so ScalarE applies exp(x/8) straight out of PSUM at full rate; no max
    subtraction is needed (scores ~ N(0,1) after the 1/sqrt(64) scale)
  - softmax normalization is deferred: O^T_unnorm accumulates over all keys,
    then rows are scaled by 1/denom (reciprocal on DVE straight from the PSUM
    denominator row, replicated across partitions by a stride-0 DMA) before
    the output projection
"""

import numpy as np

N, S, D = 2, 2048, 1024
H, HD = 16, 64
NHL = 4                 # heads per core
DH = NHL * HD           # 256 local channels
P = 128
KC = D // P             # 8 contraction chunks for the projections
SC = S // P             # 16 sequence chunks

_built = {}


def _emit(tc, out, xqt, xkt, xvt, wq, wk, wv, wp, bq, bk, stage="full"):
    from concourse import mybir

    nc = tc.nc
    f32 = mybir.dt.float32
    bf16 = mybir.dt.bfloat16
    f32r = mybir.dt.float32r
    Exp = mybir.ActivationFunctionType.Exp
    MUL = mybir.AluOpType.mult
    ADD = mybir.AluOpType.add

    with (
        tc.tile_pool(name="const", bufs=1) as cpool,
        tc.tile_pool(name="work", bufs=1) as wpool,
        tc.tile_pool(name="e", bufs=11) as epool,
        tc.tile_pool(name="small", bufs=2) as spool,
        tc.tile_pool(name="ob", bufs=4) as opool,
        tc.tile_pool(name="ps", bufs=1, space="PSUM") as ps,
    ):
        # ---------- weights / constants ----------
        wq_sb = cpool.tile([P, KC, DH], bf16)
        wk_sb = cpool.tile([P, KC, DH], bf16)
        wv_sb = cpool.tile([P, KC, DH], bf16)
        wp_sb = cpool.tile([P, 2, D], bf16)
        nc.gpsimd.dma_start(wv_sb[:], wv.rearrange("(kc p) d -> p kc d", p=P))
        nc.gpsimd.dma_start(wq_sb[:], wq.rearrange("(kc p) d -> p kc d", p=P))
        nc.gpsimd.dma_start(wk_sb[:], wk.rearrange("(kc p) d -> p kc d", p=P))
        bq_sb = cpool.tile([P, 2], f32)
        bk_sb = cpool.tile([P, 2], f32)

        # ---------- x^T loads ----------
        # xq/xk: HWDGE fp32 loads + DVE/ACT cast (HWDGE is ~40% faster per
        # byte than the SWDGE cast path); xv: SWDGE cast-DMA so both DMA
        # paths run concurrently.
        xq_sb = wpool.tile([P, KC, S], bf16)
        xk_sb = wpool.tile([P, KC, S], bf16)
        xv_sb = wpool.tile([P, KC, S], bf16)
        HS = S // 2
        warm = not stage.endswith("nowarm")
        for kc in range(KC):
            nc.gpsimd.dma_start(xv_sb[:, kc, :], xvt[kc * P:(kc + 1) * P, :])
            for x_sb, src, eng in ((xq_sb, xqt, "dve"), (xk_sb, xkt, "act")):
                for hh in range(2):
                    stg = spool.tile([P, HS], f32, tag="stg", name="stg", bufs=3)
                    nc.sync.dma_start(stg[:], src[kc * P:(kc + 1) * P,
                                                  hh * HS:(hh + 1) * HS])
                    if eng == "dve":
                        nc.vector.tensor_copy(
                            x_sb[:, kc, hh * HS:(hh + 1) * HS], stg[:])
                    else:
                        nc.scalar.copy(
                            x_sb[:, kc, hh * HS:(hh + 1) * HS], stg[:])
                    if warm:
                        # HAM keep-alive: a LDWEIGHTS paced by each chunk's
                        # cast keeps the PE activity window busy through the
                        # load phase (idle >3.4us re-throttles the PE clock
                        # to 1.2 GHz); reads bf16 data, writes nothing, and
                        # the next real matmul reloads weights anyway.
                        nc.tensor.ldweights(x_sb[:, kc, hh * HS:hh * HS + P])
        nc.gpsimd.dma_start(wp_sb[:], wp.rearrange("(c p) e -> p c e", p=P))
        if not stage.endswith("nobias"):
            nc.gpsimd.dma_start(bq_sb[:], bq.rearrange("(c p) -> p c", p=P))
            nc.gpsimd.dma_start(bk_sb[:], bk.rearrange("(c p) -> p c", p=P))

        PTAGS = ["sc0", "sc1", "av0", "av1"]

        if stage == "load":
            return

        # ---------- projections ----------
        v_sb = wpool.tile([P, SC, NHL, HD + 1], bf16)
        qt_sb = wpool.tile([P, 2, S], bf16)
        kt_sb = wpool.tile([P, 2, S], bf16)

        def emit_vproj():
            nc.vector.memset(v_sb[:], 1.0)
            for jc in range(SC):
                pv = ps.tile([P, DH], f32, tag=PTAGS[jc % 4], name="pv")
                for kc in range(KC):
                    nc.tensor.matmul(
                        pv[:],
                        lhsT=xv_sb[:, kc, jc * P:(jc + 1) * P],
                        rhs=wv_sb[:, kc, :],
                        start=(kc == 0),
                        stop=(kc == KC - 1),
                    )
                nc.vector.tensor_copy(
                    v_sb[:, jc, :, 0:HD], pv.rearrange("p (h d) -> p h d", d=HD)
                )

        def emit_qkproj():
            for x_sb, w_sb, b_sb, dst in (
                (xq_sb, wq_sb, bq_sb, qt_sb),
                (xk_sb, wk_sb, bk_sb, kt_sb),
            ):
                for c in range(2):
                    pts = [ps.tile([P, 512], f32, tag=PTAGS[ic], name=f"pts{ic}") for ic in range(4)]
                    for kc in range(KC):
                        for ic in range(4):
                            nc.tensor.matmul(
                                pts[ic][:],
                                lhsT=w_sb[:, kc, c * P:(c + 1) * P],
                                rhs=x_sb[:, kc, ic * 512:(ic + 1) * 512],
                                start=(kc == 0),
                                stop=(kc == KC - 1),
                            )
                    for ic in range(4):
                        # bias add + fp32->bf16 cast out of PSUM
                        if stage.endswith("nobias"):
                            nc.vector.tensor_copy(
                                dst[:, c, ic * 512:(ic + 1) * 512], pts[ic][:])
                        else:
                            nc.vector.tensor_scalar(
                                dst[:, c, ic * 512:(ic + 1) * 512],
                                pts[ic][:],
                                b_sb[:, c:c + 1],
                                None,
                                ADD,
                            )

        if stage == "fullv":
            emit_vproj()
            emit_qkproj()
        else:
            emit_qkproj()
            emit_vproj()

        if stage == "proj":
            return

        # ---------- attention + output projection ----------
        IH = S // 2  # queries per i-half
        for ih in range(2):
            i0 = ih * IH
            ot = wpool.tile([P, 2, IH], bf16, tag=f"ot{ih}", name=f"ot{ih}")
            for pr in range(2):  # head pair = chunk of qt/kt
                av = [ps.tile([HD + 1, IH], f32, tag=f"av{e}", name=f"av{e}") for e in range(2)]

                def emit_av(jc, ets):
                    # A@V for chunk jc, one iteration behind the scores so the
                    # (in-order) PE stream never stalls waiting on ScalarE exp
                    for e in range(2):
                        for iq in range(2):
                            nc.tensor.matmul(
                                av[e][:, iq * 512:(iq + 1) * 512],
                                lhsT=v_sb[:, jc, 2 * pr + e, :],
                                rhs=ets[e][:, iq * 512:(iq + 1) * 512],
                                start=(jc == 0),
                                stop=(jc == SC - 1),
                            )

                pending = None
                for jc in range(SC):
                    sc = [ps.tile([P, IH], f32, tag=f"sc{e}", name=f"sc{e}") for e in range(2)]
                    for e in range(2):
                        for iq in range(2):
                            nc.tensor.matmul(
                                sc[e][:, iq * 512:(iq + 1) * 512],
                                lhsT=kt_sb[HD * e:HD * (e + 1), pr, jc * P:(jc + 1) * P],
                                rhs=qt_sb[HD * e:HD * (e + 1), pr,
                                          i0 + iq * 512:i0 + (iq + 1) * 512],
                                start=True,
                                stop=True,
                            )
                    ets = []
                    for e in range(2):
                        et = epool.tile([P, IH], bf16, tag="e", name="et")
                        nc.scalar.activation(et[:], sc[e][:], Exp, scale=0.125)
                        ets.append(et)
                    if stage.endswith("nodelay"):
                        emit_av(jc, ets)
                        pending = None
                    else:
                        if pending is not None:
                            emit_av(*pending)
                        pending = (jc, ets)
                if pending is not None:
                    emit_av(*pending)
                # normalize: O^T = O^T_un * (1/denom), denom in row 64
                for e in range(2):
                    rec1 = spool.tile([HD + 1, IH], f32, tag="rec1", name="rec1")
                    nc.vector.reciprocal(rec1[HD:HD + 1, :], av[e][HD:HD + 1, :])
                    rec = spool.tile([HD, IH], f32, tag="rec", name="rec")
                    nc.sync.dma_start(
                        rec[:],
                        rec1[HD:HD + 1, None, :].to_broadcast((1, HD, IH)),
                    )
                    if e == 0:
                        nc.vector.tensor_tensor(
                            ot[0:HD, pr, :], av[e][0:HD, :], rec[:], MUL
                        )
                    else:
                        # DVE cannot write across partitions; bounce via DMA
                        otmp = spool.tile([HD, IH], bf16, tag="otmp", name="otmp")
                        nc.vector.tensor_tensor(otmp[:], av[e][0:HD, :], rec[:], MUL)
                        nc.sync.dma_start(ot[HD:P, pr, :], otmp[:])
            # output projection for query rows [i0, i0+IH)
            if stage == "attn":
                continue
            for ic8 in range(IH // P):
                r0 = i0 + ic8 * P
                for eh in range(2):
                    po = ps.tile([P, 512], f32, tag=f"av{eh}", name="po")
                    for c in range(2):
                        nc.tensor.matmul(
                            po[:],
                            lhsT=ot[:, c, ic8 * P:(ic8 + 1) * P],
                            rhs=wp_sb[:, c, eh * 512:(eh + 1) * 512],
                            start=(c == 0),
                            stop=(c == 1),
                        )
                    ob = opool.tile([P, 512], out.dtype, tag="ob", name="ob")
                    nc.vector.tensor_copy(ob[:], po[:])
                    st_eng = nc.gpsimd if stage.endswith("swst") else nc.sync
                    st_eng.dma_start(out[r0:r0 + P, eh * 512:(eh + 1) * 512], ob[:])


def _build(reps=1, stage="full"):
    key = ("nc", reps, stage)
    if key in _built:
        return _built[key]
    import concourse.tile as tile
    from concourse import bacc, mybir

    f32 = mybir.dt.float32
    nc = bacc.Bacc(
        "TRN2",
        target_bir_lowering=False,
        debug=False,
        num_devices=8,
    )
    xqt = nc.dram_tensor("xqt", [D, S], f32, kind="ExternalInput").ap()
    xkt = nc.dram_tensor("xkt", [D, S], f32, kind="ExternalInput").ap()
    xvt = nc.dram_tensor("xvt", [D, S], f32, kind="ExternalInput").ap()
    wq = nc.dram_tensor("wq", [D, DH], f32, kind="ExternalInput").ap()
    wk = nc.dram_tensor("wk", [D, DH], f32, kind="ExternalInput").ap()
    wv = nc.dram_tensor("wv", [D, DH], f32, kind="ExternalInput").ap()
    wp = nc.dram_tensor("wp", [DH, D], f32, kind="ExternalInput").ap()
    bq = nc.dram_tensor("bq", [DH], f32, kind="ExternalInput").ap()
    bk = nc.dram_tensor("bk", [DH], f32, kind="ExternalInput").ap()
    out_dt = mybir.dt.bfloat16 if stage.endswith("b16") else f32
    out = nc.dram_tensor("out", [S, D], out_dt, kind="ExternalOutput").ap()

    with tile.TileContext(nc) as tc:
        if reps == 1:
            _emit(tc, out, xqt, xkt, xvt, wq, wk, wv, wp, bq, bk, stage=stage)
        else:
            with tc.For_i(0, reps, 1):
                _emit(tc, out, xqt, xkt, xvt, wq, wk, wv, wp, bq, bk, stage=stage)
    nc.compile()
    _built[key] = nc
    return nc


def _in_maps(query, key, value, Wq, bq, Wk, bk, Wv, bv, Wp, bp):
    f = np.float32
    maps = []
    xt = {}
    for n in range(N):
        xt[n] = (
            np.ascontiguousarray(np.asarray(query, f)[n].T),
            np.ascontiguousarray(np.asarray(key, f)[n].T),
            np.ascontiguousarray(np.asarray(value, f)[n].T),
        )
    for c in range(8):
        n, g = divmod(c, 4)
        lo, hi = g * DH, (g + 1) * DH
        maps.append({
            "xqt": xt[n][0],
            "xkt": xt[n][1],
            "xvt": xt[n][2],
            "wq": np.ascontiguousarray(np.asarray(Wq, f)[:, lo:hi]),
            "wk": np.ascontiguousarray(np.asarray(Wk, f)[:, lo:hi]),
            "wv": np.ascontiguousarray(np.asarray(Wv, f)[:, lo:hi]),
            "wp": np.ascontiguousarray(np.asarray(Wp, f)[lo:hi, :]),
            "bq": np.ascontiguousarray(np.asarray(bq, f)[lo:hi]),
            "bk": np.ascontiguousarray(np.asarray(bk, f)[lo:hi]),
        })
    return maps


last_results = None  # BassKernelResults of the most recent run (for test.py)


def kernel(query, key, value, Wq, bq, Wk, bk, Wv, bv, Wp, bp, trace=False):
    global last_results
    from concourse import bass_utils

    nc = _build()
    maps = _in_maps(query, key, value, Wq, bq, Wk, bk, Wv, bv, Wp, bp)
    res = bass_utils.run_bass_kernel_spmd(
        nc, maps, core_ids=list(range(8)), trace=trace
    )
    last_results = res

    out = np.empty((N, S, D), np.float32)
    bvp = np.asarray(bv, np.float64) @ np.asarray(Wp, np.float64)
    for n in range(N):
        acc = np.zeros((S, D), np.float64)
        for g in range(4):
            acc += res.results[4 * n + g]["out"].astype(np.float64)
        acc += bvp + np.asarray(bp, np.float64)
        out[n] = acc.astype(np.float32)
    return out

